# revision 9
# baseline (speedup 1.0000x reference)
"""Trainium2 Bass kernel for nn_MultiHeadAttention_5360119185803.

Full-d_model attention (no head split) + residual + LayerNorm, B=4, T=S=2048,
E=1024, fp32 in/out.

Sharding: 8 cores; core c owns batch b=c//2 and query rows
[(c%2)*1024, (c%2+1)*1024). Each core needs the full key/value of its batch
(K/V projection duplicated across the core pair) — no collectives.

Per-core device pipeline (all matmuls in float32r = TF32-like, full PE rate):
  P1  kT = (Wk.T).T @ xk.T           [f,s]   (xk transposed on PE per block)
  P2  v  = xv @ Wv.T  -> spilled to DRAM (SBUF pressure), bias bv folded into
      bo' = bo + Wo@bv on host (attn rows sum to 1)
  P3  qT = (Wq.T/32).T @ xq.T + bq/32  [f,t]  (1/sqrt(E) folded into Wq, bq)
  P4  scoresT[s,t] = kT.T @ qT  (PSUM) -> expT = exp(scoresT)  (ACT, no
      max-subtraction: |scores/32| <~ 6 so exp is safe in fp32; bk dropped
      entirely — it shifts scores by a per-t constant, softmax-invariant)
      rowsum[1,t] = ones.T @ expT (PE), redistributed to [128,8] per-partition
  P5  ctxT[e',t] = sum_s v[s,e'] * expT[s,t]  (8 PSUM banks per t-half)
  P6  out[t,g] = (ctxT.T @ Wo.T) * (1/rowsum)[t] + bo'+ residual; LayerNorm
      over g on DVE (bn_stats/bn_aggr); gamma/beta applied only if non-trivial.

kernel() is self-contained: host prep = shard + weight transposes/scale folds.
"""

import sys

sys.path.insert(0, "/opt/trn_rl_repo")

import numpy as np

import concourse.bacc as bacc
import concourse.bass as bass
import concourse.tile as tile
from concourse import mybir
from concourse.bass_utils import run_bass_kernel_spmd
from concourse.masks import make_identity

P = 128
E = 1024          # d_model
S = 2048          # kv seq len per batch
T = 1024          # query rows per core
NE = E // P       # 8 chunks of contraction dim
NT = T // P       # 8 t tiles
NS = S // P       # 16 s tiles
FD = 512          # matmul moving free dim / PSUM bank
NBLK_S = S // FD  # 4 s blocks
NBLK_T = T // FD  # 2 t blocks

f32 = mybir.dt.float32
f32r = mybir.dt.float32r
AF = mybir.ActivationFunctionType
ALU = mybir.AluOpType

_cache = {}


def _load_weight(nc, pool, dram):
    """[E, x] fp32 DRAM -> [128, NE, x] f32r SBUF (gpsimd DMA casts+rounds)."""
    w = pool.tile([P, NE, E], f32r)
    v = dram.ap().rearrange("(j p) f -> j p f", p=P)
    for j in range(NE):
        nc.gpsimd.dma_start(out=w[:, j, :], in_=v[j])
    return w


def _transpose_block(nc, tc, xt_blk, x_dram, row0, nrows, nat_pool, tp_psum, ident):
    """Load nrows (mult of 128) rows of x [*, E] and write xt_blk[:, j, :] =
    x[row0:row0+nrows, j*128:(j+1)*128].T  (xt_blk: [128, NE, nrows] f32r)."""
    for ss in range(nrows // P):
        nat = nat_pool.tile([P, E], f32)
        nc.sync.dma_start(out=nat, in_=x_dram.ap()[row0 + ss * P: row0 + (ss + 1) * P, :])
        for j in range(NE):
            ps = tp_psum.tile([P, P], f32)
            nc.tensor.transpose(ps, nat[:, j * P:(j + 1) * P], ident)
            nc.vector.tensor_copy(xt_blk[:, j, ss * P:(ss + 1) * P], ps)


def _build(apply_gb):
    nc = bacc.Bacc("TRN2", target_bir_lowering=False, debug=False, num_devices=8)

    xq = nc.dram_tensor("xq", [T, E], f32, kind="ExternalInput")
    xk = nc.dram_tensor("xk", [S, E], f32, kind="ExternalInput")
    xv = nc.dram_tensor("xv", [S, E], f32, kind="ExternalInput")
    wqt = nc.dram_tensor("wqt", [E, E], f32, kind="ExternalInput")  # Wq.T/32 [e,f]
    wkt = nc.dram_tensor("wkt", [E, E], f32, kind="ExternalInput")  # Wk.T   [e,f]
    wvt = nc.dram_tensor("wvt", [E, E], f32, kind="ExternalInput")  # Wv.T   [e,e']
    wot = nc.dram_tensor("wot", [E, E], f32, kind="ExternalInput")  # Wo.T   [e',g]
    bq2 = nc.dram_tensor("bq2", [P, NE], f32, kind="ExternalInput")  # bq/32 tiled
    bo2 = nc.dram_tensor("bo2", [E], f32, kind="ExternalInput")      # bo + Wo@bv
    if apply_gb:
        gam = nc.dram_tensor("gam", [E], f32, kind="ExternalInput")
        bet = nc.dram_tensor("bet", [E], f32, kind="ExternalInput")
    out = nc.dram_tensor("out", [T, E], f32, kind="ExternalOutput")
    vsp = nc.dram_tensor("v_spill", [S, E], f32r)
    rs_dram = nc.dram_tensor("rs_scratch", [T], f32)

    with tile.TileContext(nc) as tc:
        consts = tc.alloc_tile_pool(name="consts", bufs=1, side="left")
        eps_t = consts.tile([P, 1], f32)
        nc.vector.memset(eps_t, 1e-6)
        ones_f = consts.tile([P, 1], f32)
        nc.vector.memset(ones_f, 1.0)
        ones_r = consts.tile([P, 1], f32r)
        nc.vector.tensor_copy(ones_r, ones_f)
        recip_t = consts.tile([P, NT], f32)

        kT_pool = tc.alloc_tile_pool(name="kT", bufs=1, side="left")
        kT = kT_pool.tile([P, NE, S], f32r)  # [f, fchunk, s] 8MB
        qT_pool = tc.alloc_tile_pool(name="qT", bufs=1, side="left")
        qT = qT_pool.tile([P, NE, T], f32r)  # [f, fchunk, t] 4MB
        identp = tc.alloc_tile_pool(name="identp", bufs=1, side="left")
        ident = identp.tile([P, P], f32)
        make_identity(nc, ident)
        bq_sb = identp.tile([P, NE], f32)
        nc.sync.dma_start(out=bq_sb, in_=bq2.ap())

        # ---- P1: kT projection ----
        with (
            tc.tile_pool(name="wk", bufs=1) as wkp,
            tc.tile_pool(name="p1nat", bufs=3) as natp,
            tc.tile_pool(name="p1xt", bufs=2) as xtp,
            tc.tile_pool(name="p1tp", bufs=4, space="PSUM") as tpp,
            tc.tile_pool(name="p1mm", bufs=3, space="PSUM") as mmp,
        ):
            wk_sb = _load_weight(nc, wkp, wkt)
            for sb in range(NBLK_S):
                xt_blk = xtp.tile([P, NE, FD], f32r)
                _transpose_block(nc, tc, xt_blk, xk, sb * FD, FD, natp, tpp, ident)
                for ft in range(NE):
                    ps = mmp.tile([P, FD], f32)
                    for j in range(NE):
                        nc.tensor.matmul(ps, wk_sb[:, j, ft * P:(ft + 1) * P],
                                         xt_blk[:, j, :],
                                         start=(j == 0), stop=(j == NE - 1))
                    nc.vector.tensor_copy(kT[:, ft, sb * FD:(sb + 1) * FD], ps)

        # ---- P2: v projection -> DRAM spill ----
        with (
            tc.tile_pool(name="wv", bufs=1) as wvp,
            tc.tile_pool(name="p2nat", bufs=3) as natp,
            tc.tile_pool(name="p2xt", bufs=2) as xtp,
            tc.tile_pool(name="p2tp", bufs=4, space="PSUM") as tpp,
            tc.tile_pool(name="p2mm", bufs=3, space="PSUM") as mmp,
            tc.tile_pool(name="p2ev", bufs=3) as evp,
        ):
            wv_sb = _load_weight(nc, wvp, wvt)
            for sb in range(NBLK_S):
                xt_blk = xtp.tile([P, NE, FD], f32r)
                _transpose_block(nc, tc, xt_blk, xv, sb * FD, FD, natp, tpp, ident)
                for ss in range(FD // P):
                    ev = evp.tile([P, E], f32r)
                    for ec in range(E // FD):
                        ps = mmp.tile([P, FD], f32)
                        for j in range(NE):
                            nc.tensor.matmul(ps, xt_blk[:, j, ss * P:(ss + 1) * P],
                                             wv_sb[:, j, ec * FD:(ec + 1) * FD],
                                             start=(j == 0), stop=(j == NE - 1))
                        nc.vector.tensor_copy(ev[:, ec * FD:(ec + 1) * FD], ps)
                    r0 = sb * FD + ss * P
                    nc.sync.dma_start(out=vsp.ap()[r0:r0 + P, :], in_=ev)

        # ---- P3: qT projection (+bq/32) ----
        with (
            tc.tile_pool(name="wq", bufs=1) as wqp,
            tc.tile_pool(name="p3nat", bufs=3) as natp,
            tc.tile_pool(name="p3xt", bufs=2) as xtp,
            tc.tile_pool(name="p3tp", bufs=4, space="PSUM") as tpp,
            tc.tile_pool(name="p3mm", bufs=3, space="PSUM") as mmp,
        ):
            wq_sb = _load_weight(nc, wqp, wqt)
            for tb in range(NBLK_T):
                xt_blk = xtp.tile([P, NE, FD], f32r)
                _transpose_block(nc, tc, xt_blk, xq, tb * FD, FD, natp, tpp, ident)
                for ft in range(NE):
                    ps = mmp.tile([P, FD], f32)
                    for j in range(NE):
                        nc.tensor.matmul(ps, wq_sb[:, j, ft * P:(ft + 1) * P],
                                         xt_blk[:, j, :],
                                         start=(j == 0), stop=(j == NE - 1))
                    nc.vector.tensor_scalar(
                        out=qT[:, ft, tb * FD:(tb + 1) * FD], in0=ps,
                        scalar1=bq_sb[:, ft:ft + 1], scalar2=None, op0=ALU.add)

        identp.release()

        # ---- P4: scoresT -> expT; rowsum -> recip ----
        ctxT_pool = tc.alloc_tile_pool(name="ctxT", bufs=1, side="right")
        ctxT = ctxT_pool.tile([P, NE, T], f32r)  # [e', echunk, t] 4MB
        expT_pool = tc.alloc_tile_pool(name="expT", bufs=1, side="right")
        expT = expT_pool.tile([P, NS, T], f32r)  # [s, stile, t] 8MB
        with tc.tile_pool(name="p4mm", bufs=4, space="PSUM") as mmp:
            for st in range(NS):
                for tb in range(NBLK_T):
                    ps = mmp.tile([P, FD], f32)
                    for j in range(NE):
                        nc.tensor.matmul(ps, kT[:, j, st * P:(st + 1) * P],
                                         qT[:, j, tb * FD:(tb + 1) * FD],
                                         start=(j == 0), stop=(j == NE - 1))
                    nc.scalar.activation(expT[:, st, tb * FD:(tb + 1) * FD], ps, AF.Exp)

        with (
            tc.tile_pool(name="p4rs", bufs=2, space="PSUM") as rsp,
            tc.tile_pool(name="p4rw", bufs=1, side="right") as rwp,
        ):
            rs_sb = rwp.tile([1, T], f32)
            for tb in range(NBLK_T):
                ps = rsp.tile([P, FD], f32)
                for st in range(NS):
                    nc.tensor.matmul(ps[0:1, :], ones_r[:, 0:1],
                                     expT[:, st, tb * FD:(tb + 1) * FD],
                                     start=(st == 0), stop=(st == NS - 1))
                nc.vector.tensor_copy(rs_sb[0:1, tb * FD:(tb + 1) * FD], ps[0:1, :])
            nc.sync.dma_start(out=rs_dram.ap(), in_=rs_sb[0:1, :])
            rsT = rwp.tile([P, NT], f32)
            nc.sync.dma_start(out=rsT, in_=rs_dram.ap().rearrange("(j p) -> p j", p=P))
            nc.vector.reciprocal(recip_t, rsT)

        qT_pool.release()
        kT_pool.release()

        # ---- P5: ctxT ----
        with (
            tc.tile_pool(name="p5v", bufs=3, side="right") as vp,
            tc.tile_pool(name="p5mm", bufs=1, space="PSUM") as mmp,
        ):
            for tb in range(NBLK_T):
                pss = [mmp.tile([P, FD], f32, name=f"ctxps{tb}_{e}",
                                tag=f"ctxps{e}") for e in range(NE)]
                for st in range(NS):
                    vt = vp.tile([P, E], f32r)
                    nc.sync.dma_start(out=vt, in_=vsp.ap()[st * P:(st + 1) * P, :])
                    for e in range(NE):
                        nc.tensor.matmul(pss[e], vt[:, e * P:(e + 1) * P],
                                         expT[:, st, tb * FD:(tb + 1) * FD],
                                         start=(st == 0), stop=(st == NS - 1))
                for e in range(NE):
                    nc.vector.tensor_copy(ctxT[:, e, tb * FD:(tb + 1) * FD], pss[e])
        expT_pool.release()

        # ---- P6: out projection + residual + LayerNorm ----
        with (
            tc.tile_pool(name="wo", bufs=1, side="right") as wop,
            tc.tile_pool(name="p6c", bufs=1, side="right") as p6c,
            tc.tile_pool(name="p6res", bufs=2, side="right") as resp,
            tc.tile_pool(name="p6y", bufs=2, side="right") as yp,
            tc.tile_pool(name="p6ln", bufs=4, side="right") as lnp,
            tc.tile_pool(name="p6out", bufs=2, side="right") as outp,
            tc.tile_pool(name="p6mm", bufs=3, space="PSUM") as mmp,
        ):
            wo_sb = _load_weight(nc, wop, wot)
            bo_sb = p6c.tile([P, E], f32)
            nc.gpsimd.dma_start(out=bo_sb, in_=bo2.ap().partition_broadcast(P))
            if apply_gb:
                gam_sb = p6c.tile([P, E], f32)
                nc.gpsimd.dma_start(out=gam_sb, in_=gam.ap().partition_broadcast(P))
                bet_sb = p6c.tile([P, E], f32)
                nc.gpsimd.dma_start(out=bet_sb, in_=bet.ap().partition_broadcast(P))
            for tt in range(NT):
                y = yp.tile([P, E], f32)
                for gc in range(E // FD):
                    ps = mmp.tile([P, FD], f32)
                    for j in range(NE):
                        nc.tensor.matmul(ps, ctxT[:, j, tt * P:(tt + 1) * P],
                                         wo_sb[:, j, gc * FD:(gc + 1) * FD],
                                         start=(j == 0), stop=(j == NE - 1))
                    nc.vector.tensor_scalar(
                        out=y[:, gc * FD:(gc + 1) * FD], in0=ps,
                        scalar1=recip_t[:, tt:tt + 1], scalar2=None, op0=ALU.mult)
                res = resp.tile([P, E], f32)
                nc.sync.dma_start(out=res, in_=xq.ap()[tt * P:(tt + 1) * P, :])
                nc.vector.tensor_add(y, y, bo_sb)
                nc.vector.tensor_add(y, y, res)
                stats = lnp.tile([P, 2, 6], f32)
                nc.vector.bn_stats(stats[:, 0, :], y[:, 0:FD])
                nc.vector.bn_stats(stats[:, 1, :], y[:, FD:E])
                mv = lnp.tile([P, 2], f32)
                nc.vector.bn_aggr(mv, stats)
                rstd = lnp.tile([P, 1], f32)
                nc.scalar.activation(rstd, mv[:, 1:2], AF.Sqrt, bias=eps_t)
                nc.vector.reciprocal(rstd, rstd)
                o = outp.tile([P, E], f32)
                nc.vector.tensor_scalar(out=o, in0=y, scalar1=mv[:, 0:1],
                                        scalar2=rstd, op0=ALU.subtract, op1=ALU.mult)
                if apply_gb:
                    nc.vector.tensor_mul(o, o, gam_sb)
                    nc.vector.tensor_add(o, o, bet_sb)
                nc.sync.dma_start(out=out.ap()[tt * P:(tt + 1) * P, :], in_=o)

        ctxT_pool.release()
        consts.release()

    nc.compile()
    return nc


def kernel(query, key, value, Wq, bq, Wk, bk, Wv, bv, Wo, bo, gamma, beta):
    query = np.asarray(query, dtype=np.float32)
    key = np.asarray(key, dtype=np.float32)
    value = np.asarray(value, dtype=np.float32)
    Wq = np.asarray(Wq, dtype=np.float32)
    bq = np.asarray(bq, dtype=np.float32)
    Wv = np.asarray(Wv, dtype=np.float32)
    bv = np.asarray(bv, dtype=np.float32)
    Wk = np.asarray(Wk, dtype=np.float32)
    Wo = np.asarray(Wo, dtype=np.float32)
    bo = np.asarray(bo, dtype=np.float32)
    gamma = np.asarray(gamma, dtype=np.float32)
    beta = np.asarray(beta, dtype=np.float32)

    scale = np.float32(1.0) / np.float32(np.sqrt(np.float32(E)))
    wqt = np.ascontiguousarray(Wq.T) * scale
    wkt = np.ascontiguousarray(Wk.T)
    wvt = np.ascontiguousarray(Wv.T)
    wot = np.ascontiguousarray(Wo.T)
    bq2 = np.ascontiguousarray((bq * scale).reshape(NE, P).T)
    bo2 = (bo + Wo @ bv).astype(np.float32)
    apply_gb = not (np.all(gamma == 1.0) and np.all(beta == 0.0))

    if apply_gb not in _cache:
        _cache[apply_gb] = _build(apply_gb)
    nc = _cache[apply_gb]

    in_maps = []
    for c in range(8):
        b, h = c // 2, c % 2
        m = {
            "xq": np.ascontiguousarray(query[b, h * T:(h + 1) * T]),
            "xk": np.ascontiguousarray(key[b]),
            "xv": np.ascontiguousarray(value[b]),
            "wqt": wqt, "wkt": wkt, "wvt": wvt, "wot": wot,
            "bq2": bq2, "bo2": bo2,
        }
        if apply_gb:
            m["gam"] = gamma
            m["bet"] = beta
        in_maps.append(m)

    global _saved_in_maps
    _saved_in_maps = in_maps
    res = run_bass_kernel_spmd(nc, in_maps, core_ids=list(range(8)))
    B = query.shape[0]
    full = np.empty((B, 2 * T, E), dtype=np.float32)
    for c in range(8):
        b, h = c // 2, c % 2
        full[b, h * T:(h + 1) * T] = res.results[c]["out"]
    return full
